# revision 1
# baseline (speedup 1.0000x reference)
"""Trainium2 Bass kernel for a 3-net MLP + masked mean-pooled cross-attention.

For each batch segment i (B=32 segments data-parallel across 8 NeuronCores):
    q/k/v = MLP3(x) per token (LeakyReLU; eval-BatchNorm folded into the
    second matmul's weights host-side), then
    emb_a[i] = mean over valid a-rows of softmax(qa kb^T / 32, key-masked) @ vb
    emb_b[i] = symmetric.

Key algebraic points exploited:
  * The mean over query rows commutes with the attention value matmul:
    emb = (sum_q w_q softmax_row_q) @ V = u @ V with u a [Lk] vector, so the
    big [Lq, D] attention-output matmul is never formed.
  * BatchNorm (eval mode) is affine -> folded into W2/b2 host-side.
  * The 1/32 score scale is folded into the q-net weights host-side.
  * Key-side masking is a rank-1 additive update (ones (x) mask-row) applied
    by one K=1 matmul into the score PSUM accumulation group; exp then
    underflows masked entries to exactly 0.
All matmul operands are bf16 with fp32 PSUM accumulation.
"""

import os
import sys

import numpy as np

for _p in ("/opt/trn_rl_repo", "/root/.axon_site/_ro/trn_rl_repo"):
    if os.path.isdir(_p) and _p not in sys.path:
        sys.path.insert(0, _p)

import ml_dtypes  # noqa: E402

B, LA, LB, D, H, P = 32, 1024, 1024, 1024, 256, 3
BN_EPS = 1e-5
SCALE = 32.0
N_CORES = 8
SEG = B // N_CORES  # segments per core
TOKBLK = 512
NEG = -1e6  # additive key mask; exp(x + NEG) underflows to exactly 0 in fp32
RAGGED = True  # specialize score loops on 128-padded lengths (host-baked)

_CACHE = {}
LAST_RESULTS = None


def _round_up(x, m):
    return (x + m - 1) // m * m


def _chunks(kpad):
    """Split [0, kpad) into free-dim chunks of <=512 (PSUM bank limit)."""
    out = []
    c = 0
    while c < kpad:
        w = min(512, kpad - c)
        out.append((c, w))
        c += w
    return out


def _build_program(sched):
    """sched[(dirn, pos)] = (n_qtiles, kpad): per segment-position loop
    structure, shared by all cores (SPMD). dirn 0: q from side a, k/v from b."""
    import concourse.bacc as bacc
    import concourse.mybir as mybir
    import concourse.tile as tile

    F32 = mybir.dt.float32
    BF16 = mybir.dt.bfloat16
    AF = mybir.ActivationFunctionType
    ALU = mybir.AluOpType
    AX = mybir.AxisListType

    nc = bacc.Bacc(
        "TRN2",
        target_bir_lowering=False,
        debug=False,
        enable_asserts=False,
        num_devices=N_CORES,
    )

    xa_d = nc.dram_tensor("xa", [SEG * LA, D], BF16, kind="ExternalInput").ap()
    xb_d = nc.dram_tensor("xb", [SEG * LB, D], BF16, kind="ExternalInput").ap()
    w1_d = nc.dram_tensor("w1", [P, D, H], BF16, kind="ExternalInput").ap()
    w2_d = nc.dram_tensor("w2", [P, H, D], BF16, kind="ExternalInput").ap()
    b1_d = nc.dram_tensor("b1", [P, H], F32, kind="ExternalInput").ap()
    b2_d = nc.dram_tensor("b2", [P, D], F32, kind="ExternalInput").ap()
    km_d = nc.dram_tensor("km", [2, SEG, LA], BF16, kind="ExternalInput").ap()
    wb_d = nc.dram_tensor("wb", [2, SEG, LA], F32, kind="ExternalInput").ap()
    o_d = nc.dram_tensor("o", [2, SEG, D], F32, kind="ExternalOutput").ap()

    DT = D // 128  # 8 d-tiles
    HT = H // 128  # 2 h-tiles
    NBLK = LA // TOKBLK  # token blocks per side

    with tile.TileContext(nc) as tc:
        with (
            tc.tile_pool(name="consts", bufs=1) as consts,
            tc.tile_pool(name="qkv", bufs=1) as qkvp,
            tc.tile_pool(name="xt", bufs=2) as xtp,
            tc.tile_pool(name="ypool", bufs=3) as ypool,
            tc.tile_pool(name="hbn", bufs=2) as hbnp,
            tc.tile_pool(name="epool", bufs=3) as epool,
            tc.tile_pool(name="stats", bufs=8) as stats,
            tc.tile_pool(name="ubc", bufs=2) as ubcp,
            tc.tile_pool(name="scratch", bufs=2) as scrp,
            tc.tile_pool(name="embp", bufs=2) as embp,
            tc.tile_pool(name="psA", bufs=2, space="PSUM") as psA,
            tc.tile_pool(name="psS", bufs=4, space="PSUM") as psS,
            tc.tile_pool(name="psU", bufs=1, space="PSUM") as psU,
            tc.tile_pool(name="dramp", bufs=2, space="DRAM") as dramp,
        ):
            # ---- constants ----
            w1_sb = []
            for dt in range(DT):
                t = consts.tile([128, P * H], BF16, name=f"w1sb{dt}")
                nc.sync.dma_start(
                    out=t,
                    in_=w1_d[:, dt * 128 : (dt + 1) * 128, :].transpose([1, 0, 2]),
                )
                w1_sb.append(t)
            w2_sb = []
            for ht in range(HT):
                t = consts.tile([128, P * D], BF16, name=f"w2sb{ht}")
                nc.sync.dma_start(
                    out=t,
                    in_=w2_d[:, ht * 128 : (ht + 1) * 128, :].transpose([1, 0, 2]),
                )
                w2_sb.append(t)
            b1_sb = consts.tile([128, P * HT], F32)
            nc.sync.dma_start(out=b1_sb, in_=b1_d.rearrange("p (t h) -> h (p t)", h=128))
            b2_sb = consts.tile([128, P * DT], F32)
            nc.sync.dma_start(out=b2_sb, in_=b2_d.rearrange("p (t d) -> d (p t)", d=128))
            km_sb = consts.tile([1, 2 * SEG * LA], BF16)
            nc.sync.dma_start(out=km_sb, in_=km_d.rearrange("a s l -> (a s l)").unsqueeze(0))
            wb_sb = consts.tile([128, 2 * SEG * 8], F32)
            nc.sync.dma_start(out=wb_sb, in_=wb_d.rearrange("a s (t p) -> p (a s t)", p=128))
            ones_sb = consts.tile([1, 128], BF16)
            nc.vector.memset(ones_sb, 1.0)

            def mlp(seg, x2d, qkv):
                """Fill qkv[p][dt]: [128, L] bf16 tiles (feature-major, partition=d)."""
                for blk in range(NBLK):
                    tok0 = seg * LA + blk * TOKBLK
                    xt = xtp.tile([128, DT, TOKBLK], BF16, tag="xt", name=f"xt{seg}{blk}")
                    for dt in range(DT):
                        nc.sync.dma_start(
                            out=xt[:, dt, :],
                            in_=x2d[tok0 : tok0 + TOKBLK, dt * 128 : (dt + 1) * 128],
                            transpose=True,
                        )
                    hbn = {}
                    for p in range(P):
                        for ht in range(HT):
                            hp = psA.tile([128, TOKBLK], F32, tag="ps_mlp", name=f"hp{seg}{blk}{p}{ht}")
                            for dt in range(DT):
                                nc.tensor.matmul(
                                    hp,
                                    w1_sb[dt][:, p * H + ht * 128 : p * H + ht * 128 + 128],
                                    xt[:, dt, :],
                                    start=(dt == 0),
                                    stop=(dt == DT - 1),
                                )
                            y = ypool.tile([128, TOKBLK], F32, tag="y", name=f"y{seg}{blk}{p}{ht}")
                            nc.scalar.activation(
                                out=y, in_=hp, func=AF.Identity,
                                bias=b1_sb[:, p * HT + ht : p * HT + ht + 1],
                            )
                            hb = hbnp.tile([128, TOKBLK], BF16, tag=f"hbn{p}{ht}", name=f"hbn{seg}{blk}{p}{ht}")
                            # LeakyReLU: max(0.01*y, y)
                            nc.vector.scalar_tensor_tensor(
                                out=hb, in0=y, scalar=0.01, in1=y,
                                op0=ALU.mult, op1=ALU.max,
                            )
                            hbn[(p, ht)] = hb
                    for p in range(P):
                        for dt in range(DT):
                            op = psA.tile([128, TOKBLK], F32, tag="ps_mlp", name=f"op{seg}{blk}{p}{dt}")
                            for ht in range(HT):
                                nc.tensor.matmul(
                                    op,
                                    w2_sb[ht][:, p * D + dt * 128 : p * D + dt * 128 + 128],
                                    hbn[(p, ht)],
                                    start=(ht == 0),
                                    stop=(ht == HT - 1),
                                )
                            nc.scalar.activation(
                                out=qkv[p][dt][:, blk * TOKBLK : (blk + 1) * TOKBLK],
                                in_=op, func=AF.Identity,
                                bias=b2_sb[:, p * DT + dt : p * DT + dt + 1],
                            )

            def attention(seg, dirn, q_tiles, k_tiles, v_tiles):
                n_qt, kpad = sched[(dirn, seg)]
                kch = _chunks(kpad)
                bd = dirn * SEG + seg
                u_ps = psU.tile([1, kpad], F32, tag="ps_u", name=f"u{bd}")

                def softmax_u(qt, s_list):
                    # negm = -rowmax over all chunks
                    nm = []
                    for i, (c0, cw) in enumerate(kch):
                        t = stats.tile([128, 1], F32, tag="negm_c", name=f"negmc{bd}_{qt}_{i}")
                        nc.vector.reduce_max(out=t, in_=s_list[i][:, :cw], axis=AX.X, negate=True)
                        nm.append(t)
                    negm = nm[0]
                    for i in range(1, len(nm)):
                        t = stats.tile([128, 1], F32, tag="negm_t", name=f"negmt{bd}_{qt}_{i}")
                        nc.vector.tensor_tensor(out=t, in0=negm, in1=nm[i], op=ALU.min)
                        negm = t
                    e = epool.tile([128, kpad], BF16, tag="e", name=f"e{bd}_{qt}")
                    zs = []
                    for i, (c0, cw) in enumerate(kch):
                        z = stats.tile([128, 1], F32, tag="z_c", name=f"z{bd}_{qt}_{i}")
                        nc.scalar.activation(
                            out=e[:, c0 : c0 + cw], in_=s_list[i][:, :cw],
                            func=AF.Exp, bias=negm, scale=1.0, accum_out=z,
                        )
                        zs.append(z)
                    ztot = zs[0]
                    for i in range(1, len(zs)):
                        t = stats.tile([128, 1], F32, tag="z_t", name=f"zt{bd}_{qt}_{i}")
                        nc.vector.tensor_tensor(out=t, in0=ztot, in1=zs[i], op=ALU.add)
                        ztot = t
                    rz = stats.tile([128, 1], F32, tag="rz", name=f"rz{bd}_{qt}")
                    nc.vector.reciprocal(out=rz, in_=ztot)
                    w = stats.tile([128, 1], BF16, tag="w", name=f"w{bd}_{qt}")
                    nc.vector.tensor_tensor(
                        out=w, in0=wb_sb[:, bd * 8 + qt : bd * 8 + qt + 1], in1=rz,
                        op=ALU.mult,
                    )
                    for i, (c0, cw) in enumerate(kch):
                        nc.tensor.matmul(
                            u_ps[:, c0 : c0 + cw], w, e[:, c0 : c0 + cw],
                            start=(qt == 0), stop=(qt == n_qt - 1),
                        )

                pend = None  # softmax of qt issued after scores of qt+1 (PE keeps busy)
                for qt in range(n_qt):
                    s_list = []
                    for ci, (c0, cw) in enumerate(kch):
                        sp = psS.tile([128, 512], F32, tag="ps_s", name=f"s{bd}_{qt}_{ci}")
                        for dt in range(DT):
                            nc.tensor.matmul(
                                sp[:, :cw],
                                q_tiles[dt][:, qt * 128 : (qt + 1) * 128],
                                k_tiles[dt][:, c0 : c0 + cw],
                                start=(dt == 0),
                                stop=False,
                            )
                        nc.tensor.matmul(
                            sp[:, :cw],
                            ones_sb,
                            km_sb[:, bd * LA + c0 : bd * LA + c0 + cw],
                            start=False,
                            stop=True,
                        )
                        s_list.append(sp)
                    if pend is not None:
                        softmax_u(*pend)
                    pend = (qt, s_list)
                softmax_u(*pend)

                # u -> SBUF -> DRAM -> partition-broadcast; emb via DVE mul-reduce
                u_sb = stats.tile([1, kpad], BF16, tag="u_sb", name=f"usb{bd}")
                for c0, cw in kch:
                    nc.scalar.activation(out=u_sb[:, c0 : c0 + cw], in_=u_ps[:, c0 : c0 + cw], func=AF.Identity)
                u_dr = dramp.tile([1, kpad], BF16, tag="u_dr", name=f"udr{bd}")
                nc.sync.dma_start(out=u_dr, in_=u_sb)
                u_bc = ubcp.tile([128, kpad], BF16, tag="u_bc", name=f"ubc{bd}")
                nc.sync.dma_start(out=u_bc, in_=u_dr[0].partition_broadcast(128))
                emb_sb = embp.tile([128, DT], F32, tag="emb", name=f"emb{bd}")
                for dt in range(DT):
                    prod = scrp.tile([128, kpad], BF16, tag="prod", name=f"prod{bd}_{dt}")
                    # (v * 1.0) * u_bc elementwise; accum_out = row-sum = emb chunk
                    nc.vector.scalar_tensor_tensor(
                        out=prod, in0=v_tiles[dt][:, :kpad], scalar=1.0, in1=u_bc,
                        op0=ALU.mult, op1=ALU.mult,
                        accum_out=emb_sb[:, dt : dt + 1],
                    )
                nc.sync.dma_start(
                    out=o_d[dirn, seg].rearrange("(t p) -> p t", p=128), in_=emb_sb
                )

            for seg in range(SEG):
                qkv_a = [
                    [qkvp.tile([128, LA], BF16, tag=f"qkva{p}{dt}", name=f"qkva{seg}_{p}_{dt}") for dt in range(DT)]
                    for p in range(P)
                ]
                qkv_b = [
                    [qkvp.tile([128, LB], BF16, tag=f"qkvb{p}{dt}", name=f"qkvb{seg}_{p}_{dt}") for dt in range(DT)]
                    for p in range(P)
                ]
                mlp(seg, xa_d, qkv_a)
                mlp(seg, xb_d, qkv_b)
                attention(seg, 0, qkv_a[0], qkv_b[1], qkv_b[2])
                attention(seg, 1, qkv_b[0], qkv_a[1], qkv_a[2])

    nc.compile()
    return nc


def _preprocess(inputs):
    """Host-side folding + sharding. Returns (sched, in_maps, perm) where
    perm[core][pos] = original segment index handled at that position."""
    a = np.asarray(inputs["a"], dtype=np.float32)
    b = np.asarray(inputs["b"], dtype=np.float32)
    W1 = np.asarray(inputs["W1"], dtype=np.float32)
    b1 = np.asarray(inputs["b1"], dtype=np.float32)
    g = np.asarray(inputs["g"], dtype=np.float32)
    bt = np.asarray(inputs["bt"], dtype=np.float32)
    rm = np.asarray(inputs["rm"], dtype=np.float32)
    rv = np.asarray(inputs["rv"], dtype=np.float32)
    W2 = np.asarray(inputs["W2"], dtype=np.float32)
    b2 = np.asarray(inputs["b2"], dtype=np.float32)
    len_a = np.asarray(inputs["len_a"], dtype=np.int64)
    len_b = np.asarray(inputs["len_b"], dtype=np.int64)

    alpha = g / np.sqrt(rv + BN_EPS)
    beta = bt - rm * alpha
    W2p = W2 * alpha[:, :, None]
    b2p = b2 + np.einsum("ph,phd->pd", beta, W2)
    W2p[0] /= SCALE  # fold 1/32 score scale into the q net
    b2p[0] /= SCALE

    bf16 = ml_dtypes.bfloat16
    w1_bf = np.ascontiguousarray(W1.astype(bf16))
    w2_bf = np.ascontiguousarray(W2p.astype(bf16))

    # Segment -> (core, position) assignment. With RAGGED, sort by score cost
    # so each position's cross-core max (which fixes the SPMD loop bounds) is
    # as small as possible.
    if RAGGED:
        order = np.argsort(-(len_a * len_b), kind="stable")
    else:
        order = np.arange(B)
    perm = [[int(order[pos * N_CORES + c]) for pos in range(SEG)] for c in range(N_CORES)]

    # per-position structure = max over cores at that position
    sched = {}
    for pos in range(SEG):
        segs = [perm[c][pos] for c in range(N_CORES)]
        for dirn in range(2):
            lq = max((len_a if dirn == 0 else len_b)[s] for s in segs)
            lk = max((len_b if dirn == 0 else len_a)[s] for s in segs)
            if not RAGGED:
                lq, lk = LA, LB
            sched[(dirn, pos)] = (
                _round_up(int(lq), 128) // 128,
                _round_up(int(lk), 128),
            )

    iota = np.arange(LA)
    in_maps = []
    for c in range(N_CORES):
        segs = perm[c]
        xa = np.ascontiguousarray(a[segs].reshape(SEG * LA, D).astype(bf16))
        xb = np.ascontiguousarray(b[segs].reshape(SEG * LB, D).astype(bf16))
        km = np.zeros((2, SEG, LA), dtype=np.float32)
        wb = np.zeros((2, SEG, LA), dtype=np.float32)
        for pos, s in enumerate(segs):
            for dirn in range(2):
                lq = int((len_a if dirn == 0 else len_b)[s])
                lk = int((len_b if dirn == 0 else len_a)[s])
                km[dirn, pos, :] = np.where(iota < lk, 0.0, NEG)
                wb[dirn, pos, :] = np.where(iota < lq, 1.0 / lq, 0.0)
        in_maps.append(
            {
                "xa": xa,
                "xb": xb,
                "w1": w1_bf,
                "w2": w2_bf,
                "b1": np.ascontiguousarray(b1),
                "b2": np.ascontiguousarray(b2p),
                "km": np.ascontiguousarray(km.astype(bf16)),
                "wb": np.ascontiguousarray(wb),
            }
        )
    return sched, in_maps, perm


def kernel(**inputs):
    global LAST_RESULTS
    from concourse.bass_utils import run_bass_kernel_spmd

    sched, in_maps, perm = _preprocess(inputs)
    key = tuple(sorted(sched.items()))
    if key not in _CACHE:
        _CACHE[key] = _build_program(sched)
    nc = _CACHE[key]

    res = run_bass_kernel_spmd(nc, in_maps, list(range(N_CORES)))
    LAST_RESULTS = res

    out = np.zeros((2, B, D), dtype=np.float32)
    for c in range(N_CORES):
        o = res.results[c]["o"]  # [2, SEG, D]
        for pos, s in enumerate(perm[c]):
            out[0, s] = o[0, pos]
            out[1, s] = o[1, pos]
    return out



# revision 7
# speedup vs baseline: 1.7869x; 1.7869x over previous
"""Trainium2 Bass kernel for a 3-net MLP + masked mean-pooled cross-attention.

B=32 segments data-parallel across 8 NeuronCores (4 per core). The eval-mode
BatchNorm folds into the second MLP layer host-side (y_p = h_p @ A_p + c_p with
h_p the post-LeakyReLU hidden), which lets everything downstream contract
through H=256 instead of D=1024:

  * scores: s = q kT / 32 = h_q (A_q A_kT/32) h_kT + row-const + 1 (x) (rk.h_k)
    with M = A_q A_kT/32 [256,256] and rk = A_k c_q/32 precomputed host-side.
    Row-constant terms are invariant under the row softmax and are dropped;
    the rk term folds into q~ = h_q M + 1 (x) rk as a rank-1 PE update.
  * values: emb = u @ v = (u @ h_v) @ A_v + c_v (sum u = 1), so the [L, D]
    q/k/v tensors are never materialized and the second MLP layer collapses
    to one [256]-vector projection per (direction, segment).
  * max |score| ~ 4, so softmax needs no row-max subtraction; exp directly
    off the score PSUM with accumulated row-sums.
  * key masking is a rank-1 additive -1e6 update (ones (x) mask-row) into the
    score PSUM; exp underflows masked entries to exactly 0.
All matmul operands are bf16 with fp32 PSUM accumulation.
"""

import os
import sys

import numpy as np

for _p in ("/opt/trn_rl_repo", "/root/.axon_site/_ro/trn_rl_repo"):
    if os.path.isdir(_p) and _p not in sys.path:
        sys.path.insert(0, _p)

import ml_dtypes  # noqa: E402

B, LA, LB, D, H, P = 32, 1024, 1024, 1024, 256, 3
BN_EPS = 1e-5
SCALE = 32.0
N_CORES = 8
SEG = B // N_CORES
TOKBLK = 512
NEG = -1e6
DT = D // 128  # 8 d-tiles
HT = H // 128  # 2 h-tiles
NDS = 2 * SEG  # direction-segment slots per core

_CACHE = {}
LAST_RESULTS = None


def _round_up(x, m):
    return (x + m - 1) // m * m


def _chunks(n):
    out, c = [], 0
    while c < n:
        w = min(TOKBLK, n - c)
        out.append((c, w))
        c += w
    return out


def _build_program(sched):
    """sched[(dirn, pos)] = (n_qt, kpad): per segment-position loop structure,
    shared by all cores (SPMD). dirn 0: q from side a, k/v from b."""
    import concourse.bacc as bacc
    import concourse.mybir as mybir
    import concourse.tile as tile

    F32 = mybir.dt.float32
    BF16 = mybir.dt.bfloat16
    AF = mybir.ActivationFunctionType
    ALU = mybir.AluOpType

    nc = bacc.Bacc(
        "TRN2",
        target_bir_lowering=False,
        debug=False,
        enable_asserts=False,
        num_devices=N_CORES,
    )

    xa_d = nc.dram_tensor("xa", [SEG * LA, D], BF16, kind="ExternalInput").ap()
    xb_d = nc.dram_tensor("xb", [SEG * LB, D], BF16, kind="ExternalInput").ap()
    w1_d = nc.dram_tensor("w1", [P, D, H], BF16, kind="ExternalInput").ap()
    b1_d = nc.dram_tensor("b1", [P, H], BF16, kind="ExternalInput").ap()
    m_d = nc.dram_tensor("m", [H, H], BF16, kind="ExternalInput").ap()
    rk_d = nc.dram_tensor("rk", [H], BF16, kind="ExternalInput").ap()
    av_d = nc.dram_tensor("av", [H, D], BF16, kind="ExternalInput").ap()
    cv_d = nc.dram_tensor("cv", [D], BF16, kind="ExternalInput").ap()
    km_d = nc.dram_tensor("km", [2, SEG, LA], BF16, kind="ExternalInput").ap()
    wb_d = nc.dram_tensor("wb", [2, SEG, LA], F32, kind="ExternalInput").ap()
    o_d = nc.dram_tensor("o", [2, SEG, D], F32, kind="ExternalOutput").ap()

    # per-position padded side lengths (side a / side b tokens needed)
    lpad = {}
    for pos in range(SEG):
        lpad[("a", pos)] = sched[(1, pos)][1]  # a is key side of dirn 1
        lpad[("b", pos)] = sched[(0, pos)][1]

    with tile.TileContext(nc) as tc:
        with (
            tc.tile_pool(name="consts", bufs=1) as consts,
            tc.tile_pool(name="xt", bufs=2) as xtp,
            tc.tile_pool(name="hp", bufs=2) as hpp,
            tc.tile_pool(name="qt", bufs=2) as qtp,
            tc.tile_pool(name="epool", bufs=3) as epool,
            tc.tile_pool(name="stats", bufs=8) as stats,
            tc.tile_pool(name="ubc", bufs=2) as ubcp,
            tc.tile_pool(name="scratch", bufs=2) as scrp,
            tc.tile_pool(name="tpool", bufs=1) as tpool,
            tc.tile_pool(name="opool", bufs=1) as opool,
            tc.tile_pool(name="psA", bufs=2, space="PSUM") as psA,
            tc.tile_pool(name="psS", bufs=2, space="PSUM") as psS,
            tc.tile_pool(name="psU", bufs=2, space="PSUM") as psU,
            tc.tile_pool(name="dramp", bufs=2, space="DRAM") as dramp,
        ):
            # ---- constants ----
            w1_sb = []
            for dt in range(DT):
                t = consts.tile([128, P * H], BF16, name=f"w1sb{dt}")
                nc.sync.dma_start(
                    out=t,
                    in_=w1_d[:, dt * 128 : (dt + 1) * 128, :].transpose([1, 0, 2]),
                )
                w1_sb.append(t)
            b1_sb = consts.tile([1, P * H], BF16)
            nc.sync.dma_start(out=b1_sb, in_=b1_d.rearrange("p h -> (p h)").unsqueeze(0))
            m_sb = consts.tile([128, HT * H], BF16)
            for hi in range(HT):
                nc.sync.dma_start(
                    out=m_sb[:, hi * H : (hi + 1) * H],
                    in_=m_d[hi * 128 : (hi + 1) * 128, :],
                )
            rk_sb = consts.tile([1, H], BF16)
            nc.sync.dma_start(out=rk_sb, in_=rk_d.unsqueeze(0))
            av_sb = consts.tile([128, HT * D], BF16)
            for hi in range(HT):
                nc.sync.dma_start(
                    out=av_sb[:, hi * D : (hi + 1) * D],
                    in_=av_d[hi * 128 : (hi + 1) * 128, :],
                )
            cv_sb = consts.tile([1, D], BF16)
            nc.sync.dma_start(out=cv_sb, in_=cv_d.unsqueeze(0))
            km_sb = consts.tile([1, 2 * SEG * LA], BF16)
            nc.sync.dma_start(out=km_sb, in_=km_d.rearrange("a s l -> (a s l)").unsqueeze(0))
            wb_sb = consts.tile([128, 2 * SEG * 8], F32)
            nc.sync.dma_start(out=wb_sb, in_=wb_d.rearrange("a s (t p) -> p (a s t)", p=128))
            ones_sb = consts.tile([1, TOKBLK], BF16)
            nc.vector.memset(ones_sb, 1.0)
            t_f32 = [tpool.tile([128, NDS], F32, name=f"tf{ht}") for ht in range(HT)]

            def mlp(seg, side, x2d, xt, h_sb):
                """h_sb: [128, 6, 1024] bf16 feature-major hidden (6 = net*2+ht)."""
                lp = lpad[(side, seg)]
                for dt in range(DT):
                    nc.sync.dma_start(
                        out=xt[:, dt, :lp],
                        in_=x2d[seg * LA : seg * LA + lp, dt * 128 : (dt + 1) * 128],
                        transpose=True,
                    )
                for g in range(P * HT):
                    for c0, cw in _chunks(lp):
                        hp = psA.tile([128, TOKBLK], F32, tag="ps_a", name=f"hp{seg}{side}{g}{c0}")
                        for dt in range(DT):
                            nc.tensor.matmul(
                                hp[:, :cw],
                                w1_sb[dt][:, g * 128 : (g + 1) * 128],
                                xt[:, dt, c0 : c0 + cw],
                                start=(dt == 0),
                                stop=False,
                            )
                        nc.tensor.matmul(
                            hp[:, :cw],
                            b1_sb[:, g * 128 : (g + 1) * 128],
                            ones_sb[:, :cw],
                            start=False,
                            stop=True,
                        )
                        # LeakyReLU (slope 0.01 per PWP table), PSUM f32 -> SBUF bf16
                        nc.scalar.activation(
                            out=h_sb[:, g, c0 : c0 + cw], in_=hp[:, :cw], func=AF.Lrelu,
                        )

            def attention(seg, dirn, h_q, h_k):
                """h_q/h_k: [128, 6, 1024] tiles of the two sides. q-net groups
                0..1 of h_q; k-net groups 2..3 and v-net groups 4..5 of h_k."""
                n_qt, kpad = sched[(dirn, seg)]
                lq = n_qt * 128
                kch = _chunks(kpad)
                bd = dirn * SEG + seg

                # q~ = h_q M + 1 (x) rk   [256, lq] feature-major bf16
                qt_sb = qtp.tile([128, HT, 1024], BF16, tag="qt", name=f"qt{bd}")
                for ho in range(HT):
                    for c0, cw in _chunks(lq):
                        qp = psA.tile([128, TOKBLK], F32, tag="ps_a", name=f"qp{bd}{ho}{c0}")
                        for hi in range(HT):
                            nc.tensor.matmul(
                                qp[:, :cw],
                                m_sb[:, hi * H + ho * 128 : hi * H + ho * 128 + 128],
                                h_q[:, hi, c0 : c0 + cw],
                                start=(hi == 0),
                                stop=False,
                            )
                        nc.tensor.matmul(
                            qp[:, :cw],
                            rk_sb[:, ho * 128 : (ho + 1) * 128],
                            ones_sb[:, :cw],
                            start=False,
                            stop=True,
                        )
                        nc.vector.tensor_copy(out=qt_sb[:, ho, c0 : c0 + cw], in_=qp[:, :cw])

                u_ps = [
                    psU.tile([1, TOKBLK], F32, tag="ps_u", name=f"u{bd}_{ci}")
                    for ci in range(len(kch))
                ]

                def softmax_u(qt, sp):
                    e = epool.tile([128, 1024], BF16, tag="e", name=f"e{bd}_{qt}")
                    z = stats.tile([128, 1], F32, tag="z", name=f"z{bd}_{qt}")
                    nc.scalar.activation(
                        out=e[:, :kpad], in_=sp[:, :kpad], func=AF.Exp, accum_out=z,
                    )
                    rz = stats.tile([128, 1], F32, tag="rz", name=f"rz{bd}_{qt}")
                    nc.vector.reciprocal(out=rz, in_=z)
                    w = stats.tile([128, 1], BF16, tag="w", name=f"w{bd}_{qt}")
                    nc.vector.tensor_tensor(
                        out=w, in0=wb_sb[:, bd * 8 + qt : bd * 8 + qt + 1], in1=rz,
                        op=ALU.mult,
                    )
                    for ci, (c0, cw) in enumerate(kch):
                        nc.tensor.matmul(
                            u_ps[ci][:, :cw], w, e[:, c0 : c0 + cw],
                            start=(qt == 0), stop=(qt == n_qt - 1),
                        )

                pend = None  # softmax of qt issued after scores of qt+1
                for qt in range(n_qt):
                    sp = psS.tile([128, 1024], F32, tag="ps_s", name=f"s{bd}_{qt}")
                    for hi in range(HT):
                        for c0, cw in kch:
                            nc.tensor.matmul(
                                sp[:, c0 : c0 + cw],
                                qt_sb[:, hi, qt * 128 : (qt + 1) * 128],
                                h_k[:, 2 + hi, c0 : c0 + cw],
                                start=(hi == 0),
                                stop=False,
                            )
                    for c0, cw in kch:
                        nc.tensor.matmul(
                            sp[:, c0 : c0 + cw],
                            ones_sb[:, :128],
                            km_sb[:, bd * LA + c0 : bd * LA + c0 + cw],
                            start=False,
                            stop=True,
                        )
                    if pend is not None:
                        softmax_u(*pend)
                    pend = (qt, sp)
                softmax_u(*pend)

                # u -> SBUF -> DRAM -> partition-broadcast; t = u . h_v via DVE
                u_sb = stats.tile([1, 1024], BF16, tag="u_sb", name=f"usb{bd}")
                for ci, (c0, cw) in enumerate(kch):
                    nc.vector.tensor_copy(out=u_sb[:, c0 : c0 + cw], in_=u_ps[ci][:, :cw])
                u_dr = dramp.tile([1, kpad], BF16, tag="u_dr", name=f"udr{bd}")
                nc.sync.dma_start(out=u_dr, in_=u_sb[:, :kpad])
                u_bc = ubcp.tile([128, 1024], BF16, tag="u_bc", name=f"ubc{bd}")
                nc.sync.dma_start(out=u_bc[:, :kpad], in_=u_dr[0].partition_broadcast(128))
                for ht in range(HT):
                    prod = scrp.tile([128, 1024], BF16, tag="prod", name=f"prod{bd}_{ht}")
                    nc.vector.scalar_tensor_tensor(
                        out=prod[:, :kpad], in0=h_k[:, 4 + ht, :kpad], scalar=1.0,
                        in1=u_bc[:, :kpad], op0=ALU.mult, op1=ALU.mult,
                        accum_out=t_f32[ht][:, bd : bd + 1],
                    )

            for seg in range(SEG):
                xt_a = xtp.tile([128, DT, 1024], BF16, tag="xta", name=f"xta{seg}")
                xt_b = xtp.tile([128, DT, 1024], BF16, tag="xtb", name=f"xtb{seg}")
                h_a = hpp.tile([128, P * HT, 1024], BF16, tag="ha", name=f"ha{seg}")
                h_b = hpp.tile([128, P * HT, 1024], BF16, tag="hb", name=f"hb{seg}")
                mlp(seg, "a", xa_d, xt_a, h_a)
                mlp(seg, "b", xb_d, xt_b, h_b)
                attention(seg, 0, h_a, h_b)
                attention(seg, 1, h_b, h_a)

            # final projection: emb[ds] = A_v^T t[ds] + c_v for all 8 ds at once
            t_bf = [tpool.tile([128, NDS], BF16, name=f"tb{ht}") for ht in range(HT)]
            for ht in range(HT):
                nc.vector.tensor_copy(out=t_bf[ht], in_=t_f32[ht])
            o_sb = opool.tile([128, NDS, DT], F32)
            for dt in range(DT):
                op = psA.tile([128, NDS], F32, tag="ps_a", name=f"op{dt}")
                for hi in range(HT):
                    nc.tensor.matmul(
                        op,
                        av_sb[:, hi * D + dt * 128 : hi * D + dt * 128 + 128],
                        t_bf[hi],
                        start=(hi == 0),
                        stop=False,
                    )
                nc.tensor.matmul(
                    op,
                    cv_sb[:, dt * 128 : (dt + 1) * 128],
                    ones_sb[:, :NDS],
                    start=False,
                    stop=True,
                )
                nc.vector.tensor_copy(out=o_sb[:, :, dt], in_=op)
            for dirn in range(2):
                for seg in range(SEG):
                    nc.sync.dma_start(
                        out=o_d[dirn, seg].rearrange("(t p) -> p t", p=128),
                        in_=o_sb[:, dirn * SEG + seg, :],
                    )

    nc.compile()
    return nc


def _preprocess(inputs):
    """Host-side folding + sharding. Returns (sched, in_maps, perm)."""
    a = np.asarray(inputs["a"], dtype=np.float32)
    b = np.asarray(inputs["b"], dtype=np.float32)
    W1 = np.asarray(inputs["W1"], dtype=np.float32)
    b1 = np.asarray(inputs["b1"], dtype=np.float32)
    g = np.asarray(inputs["g"], dtype=np.float32)
    bt = np.asarray(inputs["bt"], dtype=np.float32)
    rm = np.asarray(inputs["rm"], dtype=np.float32)
    rv = np.asarray(inputs["rv"], dtype=np.float32)
    W2 = np.asarray(inputs["W2"], dtype=np.float32)
    b2 = np.asarray(inputs["b2"], dtype=np.float32)
    len_a = np.asarray(inputs["len_a"], dtype=np.int64)
    len_b = np.asarray(inputs["len_b"], dtype=np.int64)

    alpha = g / np.sqrt(rv + BN_EPS)
    beta = bt - rm * alpha
    A = W2 * alpha[:, :, None]  # [3, H, D]
    c = np.einsum("ph,phd->pd", beta, W2) + b2  # [3, D]
    M = A[0] @ A[1].T / SCALE  # [H, H]
    rk = A[1] @ c[0] / SCALE  # [H]

    bf16 = ml_dtypes.bfloat16
    w1_bf = np.ascontiguousarray(W1.astype(bf16))
    b1_bf = np.ascontiguousarray(b1.astype(bf16))
    m_bf = np.ascontiguousarray(M.astype(bf16))
    rk_bf = np.ascontiguousarray(rk.astype(bf16))
    av_bf = np.ascontiguousarray(A[2].astype(bf16))
    cv_bf = np.ascontiguousarray(c[2].astype(bf16))

    # Segment -> (core, position): sort by score cost so each position's
    # cross-core max (which fixes the SPMD loop bounds) is small.
    order = np.argsort(-(len_a * len_b), kind="stable")
    perm = [[int(order[pos * N_CORES + cc]) for pos in range(SEG)] for cc in range(N_CORES)]

    sched = {}
    for pos in range(SEG):
        segs = [perm[cc][pos] for cc in range(N_CORES)]
        for dirn in range(2):
            lq = max((len_a if dirn == 0 else len_b)[s] for s in segs)
            lk = max((len_b if dirn == 0 else len_a)[s] for s in segs)
            sched[(dirn, pos)] = (
                _round_up(int(lq), 128) // 128,
                _round_up(int(lk), 128),
            )

    iota = np.arange(LA)
    in_maps = []
    for cc in range(N_CORES):
        segs = perm[cc]
        xa = np.ascontiguousarray(a[segs].reshape(SEG * LA, D).astype(bf16))
        xb = np.ascontiguousarray(b[segs].reshape(SEG * LB, D).astype(bf16))
        km = np.zeros((2, SEG, LA), dtype=np.float32)
        wb = np.zeros((2, SEG, LA), dtype=np.float32)
        for pos, s in enumerate(segs):
            for dirn in range(2):
                lq = int((len_a if dirn == 0 else len_b)[s])
                lk = int((len_b if dirn == 0 else len_a)[s])
                km[dirn, pos, :] = np.where(iota < lk, 0.0, NEG)
                wb[dirn, pos, :] = np.where(iota < lq, 1.0 / lq, 0.0)
        in_maps.append(
            {
                "xa": xa,
                "xb": xb,
                "w1": w1_bf,
                "b1": b1_bf,
                "m": m_bf,
                "rk": rk_bf,
                "av": av_bf,
                "cv": cv_bf,
                "km": np.ascontiguousarray(km.astype(bf16)),
                "wb": np.ascontiguousarray(wb),
            }
        )
    return sched, in_maps, perm


def kernel(**inputs):
    global LAST_RESULTS
    from concourse.bass_utils import run_bass_kernel_spmd

    sched, in_maps, perm = _preprocess(inputs)
    key = tuple(sorted(sched.items()))
    if key not in _CACHE:
        _CACHE[key] = _build_program(sched)
    nc = _CACHE[key]

    res = run_bass_kernel_spmd(nc, in_maps, list(range(N_CORES)))
    LAST_RESULTS = res

    out = np.zeros((2, B, D), dtype=np.float32)
    for cc in range(N_CORES):
        o = res.results[cc]["o"]  # [2, SEG, D]
        for pos, s in enumerate(perm[cc]):
            out[0, s] = o[0, pos]
            out[1, s] = o[1, pos]
    return out
